# revision 7
# baseline (speedup 1.0000x reference)
"""AdaptiveSoftmax forward on 8 TRN2 NeuronCores.

Strategy: data-parallel over token pairs x 2-way tensor-parallel over
vocab, no collectives.
 - cores (2p, 2p+1) share the same 1024 tokens; the even core computes
   the left vocab half, the odd core the right half (head 5001/5001,
   tails 10000/10000) -> identical SPMD graph, different input data.
 - tails: moe-style routing — band rows are compacted on host, each core
   computes proj+out matmuls only for the pair's ~410 band tokens
   (padded to a static NB), and the host scatters results into the
   zero-filled dense output (out-of-band reference rows are exactly 0).
 - all device matmuls in bf16 (inputs pre-transposed/cast/interleaved on
   host so every SBUF strip is one or two large contiguous DMAs), fp32
   PSUM accumulation, bf16 outputs upcast on host.
 - head/tail0/tail1 column groups are interleaved in emission order so
   the output-DMA rate stays under the HBM roofline while the
   TensorEngine stays busy.
 - the tiny int32 retarget vectors are pure indexing; computed on host.
"""

import os

import numpy as np
import ml_dtypes

CUTOFF = (10000, 30000, 50000)
D = 1024
VH = CUTOFF[0] + 2          # 10002
VT = CUTOFF[1] - CUTOFF[0]  # 20000 (both tails)
VHH = VH // 2               # 5001 per-core head half
VTH = VT // 2               # 10000 per-core tail half
D0 = 256
D1 = 64
N_CORES = 8
N_PAIRS = N_CORES // 2
P = 128
KT = D // P                 # 8 contraction tiles over D

BF16 = ml_dtypes.bfloat16

_graph_cache = {}


def _groups(total, tile_w, group_tiles):
    """[(col0, [tile widths]), ...] covering `total` columns."""
    tiles, c = [], 0
    while c < total:
        tiles.append(min(tile_w, total - c))
        c += tiles[-1]
    out, c = [], 0
    for i in range(0, len(tiles), group_tiles):
        chunk = tiles[i:i + group_tiles]
        out.append((c, chunk))
        c += sum(chunk)
    return out


HEAD_GROUPS = _groups(VHH, 512, 2)   # 5 groups: 4x1024 + 905
TAIL_GROUPS = _groups(VTH, 512, 4)   # 5 groups: 4x2048 + 1808


def _build_graph(TOK, NB0, NB1):
    """SPMD bacc graph. TOK tokens per pair; NB0/NB1 padded band sizes."""
    import concourse.bacc as bacc
    import concourse.tile as tile
    import concourse.mybir as mybir

    f32 = mybir.dt.float32
    bf16 = mybir.dt.bfloat16

    MT = TOK // P
    M0 = NB0 // P
    M1 = NB1 // P

    nc = bacc.Bacc("TRN2", target_bir_lowering=False, debug=False,
                   num_devices=N_CORES)

    hT_e = nc.dram_tensor("hT", (P, KT * TOK), bf16, kind="ExternalInput")
    h0T_e = nc.dram_tensor("h0T", (P, KT * NB0), bf16, kind="ExternalInput")
    h1T_e = nc.dram_tensor("h1T", (P, KT * NB1), bf16, kind="ExternalInput")
    hw_e = nc.dram_tensor("head_wI", (P, KT * VHH), bf16, kind="ExternalInput")
    p0w_e = nc.dram_tensor("proj0_wI", (P, KT * D0), bf16, kind="ExternalInput")
    p1w_e = nc.dram_tensor("proj1_wI", (P, KT * D1), bf16, kind="ExternalInput")
    o0w_e = nc.dram_tensor("out0_wI", (P, 2 * VTH), bf16, kind="ExternalInput")
    o1w_e = nc.dram_tensor("out1_wT", (D1, VTH), bf16, kind="ExternalInput")
    ho_e = nc.dram_tensor("head_o", (TOK, VHH), bf16, kind="ExternalOutput")
    t0_e = nc.dram_tensor("t0_o", (NB0, VTH), bf16, kind="ExternalOutput")
    t1_e = nc.dram_tensor("t1_o", (NB1, VTH), bf16, kind="ExternalOutput")

    copy_ctr = [0]

    with tile.TileContext(nc) as tc:
        with (
            tc.tile_pool(name="res", bufs=1) as rpool,
            tc.tile_pool(name="wstrip", bufs=5) as wpool,
            tc.tile_pool(name="ostage", bufs=10) as opool,
            tc.tile_pool(name="psmain", bufs=6, space="PSUM") as psm,
            tc.tile_pool(name="psproj", bufs=1, space="PSUM") as psp,
        ):
            def evict(dst, src):
                if copy_ctr[0] % 2 == 0:
                    nc.scalar.copy(dst, src)
                else:
                    nc.vector.tensor_copy(dst, src)
                copy_ctr[0] += 1

            def dma_split(dst, src, n):
                F = dst.shape[-1]
                step = -(-F // n)
                j = 0
                while j < F:
                    w = min(step, F - j)
                    nc.sync.dma_start(dst[:, j:j + w], src[:, j:j + w])
                    j += w

            # ---- resident loads; proj inputs first so proj can start ----
            p0w = rpool.tile([P, KT * D0], bf16, tag="p0w")
            nc.sync.dma_start(p0w[:], p0w_e.ap()[:])
            h0T = rpool.tile([P, KT * NB0], bf16, tag="h0T")
            dma_split(h0T[:], h0T_e.ap()[:], 2)
            p1w = rpool.tile([P, KT * D1], bf16, tag="p1w")
            nc.sync.dma_start(p1w[:], p1w_e.ap()[:])
            h1T = rpool.tile([P, KT * NB1], bf16, tag="h1T")
            dma_split(h1T[:], h1T_e.ap()[:], 2)
            hT = rpool.tile([P, KT * TOK], bf16, tag="hT")
            dma_split(hT[:], hT_e.ap()[:], 4)

            # ---- proj0: p0T[d0, NB0] (2 partition tiles) ----
            p0T = rpool.tile([P, 2 * NB0], bf16, tag="p0T")
            for mp in range(2):
                ps = psp.tile([P, NB0], f32, tag="ps0")
                for k in range(KT):
                    nc.tensor.matmul(
                        ps[:],
                        p0w[:, k * D0 + mp * P: k * D0 + (mp + 1) * P],
                        h0T[:, k * NB0:(k + 1) * NB0],
                        start=(k == 0), stop=(k == KT - 1),
                    )
                evict(p0T[:, mp * NB0:(mp + 1) * NB0], ps[:])

            # ---- proj1: p1T[d1, NB1] ----
            p1T = rpool.tile([D1, NB1], bf16, tag="p1T")
            ps = psp.tile([D1, NB1], f32, tag="ps1")
            for k in range(KT):
                nc.tensor.matmul(
                    ps[:],
                    p1w[:, k * D1:(k + 1) * D1],
                    h1T[:, k * NB1:(k + 1) * NB1],
                    start=(k == 0), stop=(k == KT - 1),
                )
            evict(p1T[:], ps[:])

            # ---- one column group of an output matmul ----
            def out_group(c0, tiles, KP, strip_part, w_e, w_il, out_e, M,
                          lhsT_fn, strip_splits):
                W = sum(tiles)
                strip = wpool.tile([strip_part, KP * W], bf16, tag="ws")
                if w_il:
                    dma_split(strip[:], w_e.ap()[:, KP * c0: KP * (c0 + W)],
                              strip_splits)
                else:
                    dma_split(strip[:], w_e.ap()[:, c0:c0 + W], strip_splits)
                for m in range(M):
                    ot = opool.tile([P, W], bf16, tag="o")
                    j = 0
                    for nsz in tiles:
                        ps = psm.tile([P, nsz], f32, tag="ps")
                        for kp in range(KP):
                            nc.tensor.matmul(
                                ps[:],
                                lhsT_fn(kp, m),
                                strip[:, kp * W + j: kp * W + j + nsz],
                                start=(kp == 0), stop=(kp == KP - 1),
                            )
                        evict(ot[:, j:j + nsz], ps[:])
                        j += nsz
                    dma_split(out_e.ap()[m * P:(m + 1) * P, c0:c0 + W],
                              ot[:], 2)

            # ---- interleave tail1 / tail0 / head column groups ----
            # (tails first: their inputs are small, so the PE starts while
            # the big head strips are still streaming in)
            n_iter = max(len(HEAD_GROUPS), len(TAIL_GROUPS))
            for i in range(n_iter):
                if i < len(TAIL_GROUPS):
                    c0, tiles = TAIL_GROUPS[i]
                    out_group(
                        c0, tiles, 1, D1, o1w_e, False, t1_e, M1,
                        lambda k, m: p1T[:, m * P:(m + 1) * P],
                        2)
                    out_group(
                        c0, tiles, 2, P, o0w_e, True, t0_e, M0,
                        lambda k, m: p0T[:, k * NB0 + m * P: k * NB0 + (m + 1) * P],
                        4)
                if i < len(HEAD_GROUPS):
                    c0, tiles = HEAD_GROUPS[i]
                    out_group(
                        c0, tiles, KT, P, hw_e, True, ho_e, MT,
                        lambda k, m: hT[:, k * TOK + m * P: k * TOK + (m + 1) * P],
                        4)

    nc.compile()
    return nc


def _pad128(n):
    return max(P, ((n + P - 1) // P) * P)


def _install_ntff_hook():
    """Register the NTFF profile hook the agent image's antenv lacks, so
    run_bass_kernel_spmd(trace=True) can report exec_time_ns under axon."""
    import sys
    import types
    if 'antenv.axon_hooks' in sys.modules:
        return
    mod = types.ModuleType('antenv.axon_hooks')
    mod._hook = None
    mod.set_axon_ntff_profile_hook = lambda h: setattr(mod, '_hook', h)
    mod.get_axon_ntff_profile_hook = lambda: mod._hook
    sys.modules['antenv.axon_hooks'] = mod
    import antenv
    antenv.axon_hooks = mod
    from trn_agent_boot.trn_boot import _ntff_profile_via_ctypes
    mod._hook = _ntff_profile_via_ctypes('/opt/axon/libaxon_pjrt.so')
    import concourse.bass_utils as bu
    bu.upload_artifacts = lambda tmpdir: f"local:{tmpdir}"


def _run_spmd(nc, in_maps, profile):
    from concourse.bass_utils import run_bass_kernel_spmd
    kwargs = {}
    if profile:
        kwargs["trace"] = True
    return run_bass_kernel_spmd(nc, in_maps, core_ids=list(range(N_CORES)),
                                **kwargs)


def _interleave_k(a_t, kt):
    """[kt*P, F] -> [P, kt*F] with X[p, k*F + j] = a_t[k*P + p, j]."""
    ktp, F = a_t.shape
    assert ktp == kt * P
    return np.ascontiguousarray(
        a_t.reshape(kt, P, F).transpose(1, 0, 2).reshape(P, kt * F))


def _interleave_groups(a_t, kt, groups):
    """Group-blocked interleave: for each (c0, tiles) with width W, block
    [P, kt*W] with X[p, k*W + j] = a_t[k*P + p, c0 + j]; blocks concat."""
    A = a_t.reshape(kt, P, a_t.shape[1])
    blocks = []
    for c0, tiles in groups:
        W = sum(tiles)
        blocks.append(A[:, :, c0:c0 + W].transpose(1, 0, 2).reshape(P, kt * W))
    return np.ascontiguousarray(np.concatenate(blocks, axis=1))


def kernel(input, target, head_w, proj0_w, out0_w, proj1_w, out1_w):
    c0_, c1_, c2_ = CUTOFF
    h = np.ascontiguousarray(np.asarray(input, dtype=np.float32)).reshape(-1, D)
    t = np.asarray(target, dtype=np.int32).reshape(-1)
    N = h.shape[0]
    assert N % N_CORES == 0
    TOK = N // N_PAIRS          # tokens per core pair

    m0 = (t >= c0_) & (t < c1_)
    m1 = (t >= c1_) & (t < c2_)

    # int32 retarget outputs (pure indexing, negligible)
    new_t_head = np.where(m0, c0_, np.where(m1, c0_ + 1, t)).astype(np.int32)
    new_t0 = np.where(m0, t - c0_, 0).astype(np.int32)
    new_t1 = np.where(m1, t - c1_, 0).astype(np.int32)

    # per-pair routing of band rows
    idx0 = [np.nonzero(m0[p * TOK:(p + 1) * TOK])[0] for p in range(N_PAIRS)]
    idx1 = [np.nonzero(m1[p * TOK:(p + 1) * TOK])[0] for p in range(N_PAIRS)]
    NB0 = _pad128(max(len(i) for i in idx0))
    NB1 = _pad128(max(len(i) for i in idx1))

    key = (TOK, NB0, NB1)
    if key not in _graph_cache:
        _graph_cache[key] = _build_graph(TOK, NB0, NB1)
    nc = _graph_cache[key]

    bf = lambda a: a.astype(BF16)

    hw_t = np.asarray(head_w, np.float32).T      # [D, VH]
    o0_t = np.asarray(out0_w, np.float32).T      # [D0, VT]
    o1_t = np.asarray(out1_w, np.float32).T      # [D1, VT]
    halves = []
    for j in range(2):
        halves.append({
            "head_wI": bf(_interleave_groups(
                np.ascontiguousarray(hw_t[:, j * VHH:(j + 1) * VHH]),
                KT, HEAD_GROUPS)),
            "out0_wI": bf(_interleave_groups(
                np.ascontiguousarray(o0_t[:, j * VTH:(j + 1) * VTH]),
                2, TAIL_GROUPS)),
            "out1_wT": bf(np.ascontiguousarray(o1_t[:, j * VTH:(j + 1) * VTH])),
        })
    w_common = {
        "proj0_wI": bf(_interleave_k(np.ascontiguousarray(
            np.asarray(proj0_w, np.float32).T), KT)),
        "proj1_wI": bf(_interleave_k(np.ascontiguousarray(
            np.asarray(proj1_w, np.float32).T), KT)),
    }

    in_maps = []
    for p in range(N_PAIRS):
        hc = h[p * TOK:(p + 1) * TOK]
        h0 = np.zeros((NB0, D), np.float32)
        h0[:len(idx0[p])] = hc[idx0[p]]
        h1 = np.zeros((NB1, D), np.float32)
        h1[:len(idx1[p])] = hc[idx1[p]]
        acts = {
            "hT": bf(_interleave_k(np.ascontiguousarray(hc.T), KT)),
            "h0T": bf(_interleave_k(np.ascontiguousarray(h0.T), KT)),
            "h1T": bf(_interleave_k(np.ascontiguousarray(h1.T), KT)),
        }
        for j in range(2):
            in_maps.append({**acts, **w_common, **halves[j]})

    profile = bool(os.environ.get("BASS_KERNEL_PROFILE"))
    if profile:
        try:
            _install_ntff_hook()
        except Exception:
            pass
    res = _run_spmd(nc, in_maps, profile)
    if profile and res.exec_time_ns is not None:
        print(f"HW exec time: {res.exec_time_ns} ns")
        kernel.last_exec_time_ns = res.exec_time_ns

    head_out = np.empty((N, VH), np.float32)
    tail0 = np.zeros((N, VT), np.float32)
    tail1 = np.zeros((N, VT), np.float32)
    for p in range(N_PAIRS):
        base = p * TOK
        for j in range(2):
            r = res.results[2 * p + j]
            head_out[base:base + TOK, j * VHH:(j + 1) * VHH] = \
                r["head_o"].astype(np.float32)
            if len(idx0[p]):
                tail0[base + idx0[p], j * VTH:(j + 1) * VTH] = \
                    r["t0_o"][:len(idx0[p])].astype(np.float32)
            if len(idx1[p]):
                tail1[base + idx1[p], j * VTH:(j + 1) * VTH] = \
                    r["t1_o"][:len(idx1[p])].astype(np.float32)

    return (head_out, tail0, tail1, new_t_head, new_t0, new_t1)


# revision 30
# speedup vs baseline: 1.2072x; 1.2072x over previous
"""AdaptiveSoftmax forward on 8 TRN2 NeuronCores.

Strategy: data-parallel over token pairs x 2-way tensor-parallel over
vocab, no collectives.
 - cores (2p, 2p+1) share the same 1024 tokens; the even core computes
   the left vocab half, the odd core the right half (head 5001/5001,
   tails 10000/10000) -> identical SPMD graph, different input data.
 - tails: moe-style routing — band rows are compacted on host, each core
   computes proj+out matmuls only for the pair's ~410 band tokens
   (padded to a static NB), and the host scatters results into the
   zero-filled dense output (out-of-band reference rows are exactly 0).
 - all device matmuls in bf16 (inputs pre-transposed/cast/interleaved on
   host so every SBUF strip is one or two large contiguous DMAs), fp32
   PSUM accumulation, bf16 outputs upcast on host.
 - head/tail0/tail1 column groups are interleaved in emission order so
   the output-DMA rate stays under the HBM roofline while the
   TensorEngine stays busy.
 - the tiny int32 retarget vectors are pure indexing; computed on host.
"""

import os

import numpy as np
import ml_dtypes

CUTOFF = (10000, 30000, 50000)
D = 1024
VH = CUTOFF[0] + 2          # 10002
VT = CUTOFF[1] - CUTOFF[0]  # 20000 (both tails)
VHH = VH // 2               # 5001 per-core head half
VTH = VT // 2               # 10000 per-core tail half
D0 = 256
D1 = 64
N_CORES = 8
N_PAIRS = N_CORES // 2
P = 128
KT = D // P                 # 8 contraction tiles over D

BF16 = ml_dtypes.bfloat16

_graph_cache = {}


def _groups(total, tile_w, group_tiles):
    """[(col0, [tile widths]), ...] covering `total` columns."""
    tiles, c = [], 0
    while c < total:
        tiles.append(min(tile_w, total - c))
        c += tiles[-1]
    out, c = [], 0
    for i in range(0, len(tiles), group_tiles):
        chunk = tiles[i:i + group_tiles]
        out.append((c, chunk))
        c += sum(chunk)
    return out


HEAD_GROUPS = _groups(VHH, 512, 2)   # 5 groups: 4x1024 + 905
TAIL_GROUPS = _groups(VTH, 512, 4)   # 5 groups: 4x2048 + 1808


def _build_graph(TOK, NB0, NB1):
    """SPMD bacc graph. TOK tokens per pair; NB0/NB1 padded band sizes."""
    import concourse.bacc as bacc
    import concourse.tile as tile
    import concourse.mybir as mybir

    f32 = mybir.dt.float32
    bf16 = mybir.dt.bfloat16

    MT = TOK // P
    M0 = NB0 // P
    M1 = NB1 // P

    nc = bacc.Bacc("TRN2", target_bir_lowering=False, debug=False,
                   num_devices=N_CORES)

    hT_e = nc.dram_tensor("hT", (P, KT * TOK), bf16, kind="ExternalInput")
    h0T_e = nc.dram_tensor("h0T", (P, KT * NB0), bf16, kind="ExternalInput")
    h1T_e = nc.dram_tensor("h1T", (P, KT * NB1), bf16, kind="ExternalInput")
    hw_e = nc.dram_tensor("head_wI", (P, KT * VHH), bf16, kind="ExternalInput")
    p0w_e = nc.dram_tensor("proj0_wI", (P, KT * D0), bf16, kind="ExternalInput")
    p1w_e = nc.dram_tensor("proj1_wI", (P, KT * D1), bf16, kind="ExternalInput")
    o0w_e = nc.dram_tensor("out0_wI", (P, 2 * VTH), bf16, kind="ExternalInput")
    # out1 rows duplicated into both partition halves for row-packed matmuls
    o1w_e = nc.dram_tensor("out1_wD", (P, VTH), bf16, kind="ExternalInput")
    ho_e = nc.dram_tensor("head_o", (TOK, VHH), bf16, kind="ExternalOutput")
    t0_e = nc.dram_tensor("t0_o", (NB0, VTH), bf16, kind="ExternalOutput")
    t1_e = nc.dram_tensor("t1_o", (NB1, VTH), bf16, kind="ExternalOutput")

    copy_ctr = [0]

    with tile.TileContext(nc) as tc:
        with (
            tc.tile_pool(name="res", bufs=1) as rpool,
            tc.tile_pool(name="wstrip", bufs=5) as wpool,
            tc.tile_pool(name="ostage", bufs=10) as opool,
            tc.tile_pool(name="psmain", bufs=6, space="PSUM") as psm,
            tc.tile_pool(name="psproj", bufs=1, space="PSUM") as psp,
        ):
            def evict(dst, src):
                if copy_ctr[0] % 2 == 0:
                    nc.scalar.copy(dst, src)
                else:
                    nc.vector.tensor_copy(dst, src)
                copy_ctr[0] += 1

            def dma_split(dst, src, n):
                F = dst.shape[-1]
                step = -(-F // n)
                j = 0
                while j < F:
                    w = min(step, F - j)
                    nc.sync.dma_start(dst[:, j:j + w], src[:, j:j + w])
                    j += w

            # ---- PE warm-up: dummy matmuls on a zeroed tile keep the
            # TensorEngine active (HAM un-throttled) while the first
            # input DMAs stream in; results are never read ----
            warm = rpool.tile([P, 512], bf16, tag="warm")
            nc.vector.memset(warm[:], 0.0)
            wps = psm.tile([P, 512], f32, tag="ps", name="warm_ps")
            N_WARM = 56
            for i in range(N_WARM):
                nc.tensor.matmul(warm_out := wps[:], warm[:, 0:P], warm[:],
                                 start=(i == 0), stop=(i == N_WARM - 1))

            # ---- resident loads; proj inputs first so proj can start ----
            p0w = rpool.tile([P, KT * D0], bf16, tag="p0w")
            nc.sync.dma_start(p0w[:], p0w_e.ap()[:])
            h0T = rpool.tile([P, KT * NB0], bf16, tag="h0T")
            dma_split(h0T[:], h0T_e.ap()[:], 2)
            p1w = rpool.tile([P, KT * D1], bf16, tag="p1w")
            nc.sync.dma_start(p1w[:], p1w_e.ap()[:])
            h1T = rpool.tile([P, KT * NB1], bf16, tag="h1T")
            dma_split(h1T[:], h1T_e.ap()[:], 2)
            hT = rpool.tile([P, KT * TOK], bf16, tag="hT")
            dma_split(hT[:], hT_e.ap()[:], 4)

            # ---- proj0: p0T[d0, NB0] (2 partition tiles) ----
            p0T = rpool.tile([P, 2 * NB0], bf16, tag="p0T")
            for mp in range(2):
                ps = psp.tile([P, NB0], f32, tag="ps0")
                for k in range(KT):
                    nc.tensor.matmul(
                        ps[:],
                        p0w[:, k * D0 + mp * P: k * D0 + (mp + 1) * P],
                        h0T[:, k * NB0:(k + 1) * NB0],
                        start=(k == 0), stop=(k == KT - 1),
                    )
                evict(p0T[:, mp * NB0:(mp + 1) * NB0], ps[:])

            # ---- proj1: p1T[d1, NB1], duplicated into both partition halves
            p1T = rpool.tile([P, NB1], bf16, tag="p1T")
            ps = psp.tile([D1, NB1], f32, tag="ps1")
            for k in range(KT):
                nc.tensor.matmul(
                    ps[:],
                    p1w[:, k * D1:(k + 1) * D1],
                    h1T[:, k * NB1:(k + 1) * NB1],
                    start=(k == 0), stop=(k == KT - 1),
                )
            evict(p1T[0:D1, :], ps[:])
            evict(p1T[D1:P, :], ps[:])

            # ---- one column group of an output matmul ----
            def out_group(c0, tiles, KP, strip_part, w_e, w_il, out_e, M,
                          lhsT_fn, strip_splits, stag="ws"):
                W = sum(tiles)
                strip = wpool.tile([strip_part, KP * W], bf16, tag=stag,
                                   name=f"strip_{stag}_{c0}",
                                   bufs=3 if stag == "wsh" else 4)
                if w_il:
                    dma_split(strip[:], w_e.ap()[:, KP * c0: KP * (c0 + W)],
                              strip_splits)
                else:
                    dma_split(strip[:], w_e.ap()[:, c0:c0 + W], strip_splits)
                for m in range(M):
                    ot = opool.tile([P, W], bf16, tag="o")
                    j = 0
                    for nsz in tiles:
                        ps = psm.tile([P, nsz], f32, tag="ps")
                        for kp in range(KP):
                            nc.tensor.matmul(
                                ps[:],
                                lhsT_fn(kp, m),
                                strip[:, kp * W + j: kp * W + j + nsz],
                                start=(kp == 0), stop=(kp == KP - 1),
                            )
                        evict(ot[:, j:j + nsz], ps[:])
                        j += nsz
                    dma_split(out_e.ap()[m * P:(m + 1) * P, c0:c0 + W],
                              ot[:], 2)

            # ---- tail1 group: K=64, two m-tiles packed into the array ----
            def t1_group(c0, tiles):
                W = sum(tiles)
                strip = wpool.tile([P, W], bf16, tag="ws",
                                   name=f"strip_t1_{c0}", bufs=4)
                dma_split(strip[:], o1w_e.ap()[:, c0:c0 + W], 2)
                for mp in range(0, M1 - (M1 % 2), 2):
                    ot_a = opool.tile([P, W], bf16, tag="o", name=f"ot1a_{c0}_{mp}")
                    ot_b = opool.tile([P, W], bf16, tag="o", name=f"ot1b_{c0}_{mp}")
                    j = 0
                    for nsz in tiles:
                        ps_a = psm.tile([P, nsz], f32, tag="ps", name=f"pt1a_{c0}_{mp}_{j}")
                        ps_b = psm.tile([P, nsz], f32, tag="ps", name=f"pt1b_{c0}_{mp}_{j}")
                        nc.tensor.matmul(
                            ps_a[:], p1T[0:D1, mp * P:(mp + 1) * P],
                            strip[0:D1, j:j + nsz], start=True, stop=True)
                        nc.tensor.matmul(
                            ps_b[:], p1T[D1:P, (mp + 1) * P:(mp + 2) * P],
                            strip[D1:P, j:j + nsz], start=True, stop=True)
                        evict(ot_a[:, j:j + nsz], ps_a[:])
                        evict(ot_b[:, j:j + nsz], ps_b[:])
                        j += nsz
                    dma_split(t1_e.ap()[mp * P:(mp + 1) * P, c0:c0 + W],
                              ot_a[:], 2)
                    dma_split(t1_e.ap()[(mp + 1) * P:(mp + 2) * P, c0:c0 + W],
                              ot_b[:], 2)
                if M1 % 2:
                    m = M1 - 1
                    ot = opool.tile([P, W], bf16, tag="o", name=f"ot1c_{c0}")
                    j = 0
                    for nsz in tiles:
                        ps = psm.tile([P, nsz], f32, tag="ps", name=f"pt1c_{c0}_{j}")
                        nc.tensor.matmul(
                            ps[:], p1T[0:D1, m * P:(m + 1) * P],
                            strip[0:D1, j:j + nsz], start=True, stop=True)
                        evict(ot[:, j:j + nsz], ps[:])
                        j += nsz
                    dma_split(t1_e.ap()[m * P:(m + 1) * P, c0:c0 + W],
                              ot[:], 2)

            # ---- interleave column groups: head leads each round (its
            # strips are prefetched and it is PE-dense, absorbing the
            # DMA bursts of the tail groups that follow) ----
            def head_group(i):
                c0, tiles = HEAD_GROUPS[i]
                out_group(
                    c0, tiles, KT, P, hw_e, True, ho_e, MT,
                    lambda k, m: hT[:, k * TOK + m * P: k * TOK + (m + 1) * P],
                    4, stag="wsh")

            def tail_groups(i):
                c0, tiles = TAIL_GROUPS[i]
                t1_group(c0, tiles)
                out_group(
                    c0, tiles, 2, P, o0w_e, True, t0_e, M0,
                    lambda k, m: p0T[:, k * NB0 + m * P: k * NB0 + (m + 1) * P],
                    4)

            # H0 T0 H1 T1 H2 T2 H3 T3 T4 H4 — starts and ends PE-dense so
            # the tail groups' output-DMA bursts overlap head compute.
            nH, nT = len(HEAD_GROUPS), len(TAIL_GROUPS)
            head_group(0)
            for i in range(1, nH - 1):
                tail_groups(i - 1)
                head_group(i)
            for i in range(nH - 2, nT):
                tail_groups(i)
            head_group(nH - 1)

    nc.compile()
    return nc


def _pad128(n):
    return max(P, ((n + P - 1) // P) * P)


def _install_ntff_hook():
    """Register the NTFF profile hook the agent image's antenv lacks, so
    run_bass_kernel_spmd(trace=True) can report exec_time_ns under axon."""
    import sys
    import types
    if 'antenv.axon_hooks' in sys.modules:
        return
    mod = types.ModuleType('antenv.axon_hooks')
    mod._hook = None
    mod.set_axon_ntff_profile_hook = lambda h: setattr(mod, '_hook', h)
    mod.get_axon_ntff_profile_hook = lambda: mod._hook
    sys.modules['antenv.axon_hooks'] = mod
    import antenv
    antenv.axon_hooks = mod
    from trn_agent_boot.trn_boot import _ntff_profile_via_ctypes
    mod._hook = _ntff_profile_via_ctypes('/opt/axon/libaxon_pjrt.so')
    import concourse.bass_utils as bu
    bu.upload_artifacts = lambda tmpdir: f"local:{tmpdir}"


def _run_spmd(nc, in_maps, profile):
    from concourse.bass_utils import run_bass_kernel_spmd
    kwargs = {}
    if profile:
        kwargs["trace"] = True
    return run_bass_kernel_spmd(nc, in_maps, core_ids=list(range(N_CORES)),
                                **kwargs)


def _interleave_k(a_t, kt):
    """[kt*P, F] -> [P, kt*F] with X[p, k*F + j] = a_t[k*P + p, j]."""
    ktp, F = a_t.shape
    assert ktp == kt * P
    return np.ascontiguousarray(
        a_t.reshape(kt, P, F).transpose(1, 0, 2).reshape(P, kt * F))


def _interleave_groups(a_t, kt, groups):
    """Group-blocked interleave: for each (c0, tiles) with width W, block
    [P, kt*W] with X[p, k*W + j] = a_t[k*P + p, c0 + j]; blocks concat."""
    A = a_t.reshape(kt, P, a_t.shape[1])
    blocks = []
    for c0, tiles in groups:
        W = sum(tiles)
        blocks.append(A[:, :, c0:c0 + W].transpose(1, 0, 2).reshape(P, kt * W))
    return np.ascontiguousarray(np.concatenate(blocks, axis=1))


def kernel(input, target, head_w, proj0_w, out0_w, proj1_w, out1_w):
    c0_, c1_, c2_ = CUTOFF
    h = np.ascontiguousarray(np.asarray(input, dtype=np.float32)).reshape(-1, D)
    t = np.asarray(target, dtype=np.int32).reshape(-1)
    N = h.shape[0]
    assert N % N_CORES == 0
    TOK = N // N_PAIRS          # tokens per core pair

    m0 = (t >= c0_) & (t < c1_)
    m1 = (t >= c1_) & (t < c2_)

    # int32 retarget outputs (pure indexing, negligible)
    new_t_head = np.where(m0, c0_, np.where(m1, c0_ + 1, t)).astype(np.int32)
    new_t0 = np.where(m0, t - c0_, 0).astype(np.int32)
    new_t1 = np.where(m1, t - c1_, 0).astype(np.int32)

    # per-pair routing of band rows
    idx0 = [np.nonzero(m0[p * TOK:(p + 1) * TOK])[0] for p in range(N_PAIRS)]
    idx1 = [np.nonzero(m1[p * TOK:(p + 1) * TOK])[0] for p in range(N_PAIRS)]
    NB0 = _pad128(max(len(i) for i in idx0))
    NB1 = _pad128(max(len(i) for i in idx1))

    key = (TOK, NB0, NB1)
    if key not in _graph_cache:
        _graph_cache[key] = _build_graph(TOK, NB0, NB1)
    nc = _graph_cache[key]

    bf = lambda a: a.astype(BF16)

    hw_t = np.asarray(head_w, np.float32).T      # [D, VH]
    o0_t = np.asarray(out0_w, np.float32).T      # [D0, VT]
    o1_t = np.asarray(out1_w, np.float32).T      # [D1, VT]
    halves = []
    for j in range(2):
        halves.append({
            "head_wI": bf(_interleave_groups(
                np.ascontiguousarray(hw_t[:, j * VHH:(j + 1) * VHH]),
                KT, HEAD_GROUPS)),
            "out0_wI": bf(_interleave_groups(
                np.ascontiguousarray(o0_t[:, j * VTH:(j + 1) * VTH]),
                2, TAIL_GROUPS)),
            "out1_wD": bf(np.ascontiguousarray(np.concatenate(
                [o1_t[:, j * VTH:(j + 1) * VTH]] * 2, axis=0))),
        })
    w_common = {
        "proj0_wI": bf(_interleave_k(np.ascontiguousarray(
            np.asarray(proj0_w, np.float32).T), KT)),
        "proj1_wI": bf(_interleave_k(np.ascontiguousarray(
            np.asarray(proj1_w, np.float32).T), KT)),
    }

    in_maps = []
    for p in range(N_PAIRS):
        hc = h[p * TOK:(p + 1) * TOK]
        h0 = np.zeros((NB0, D), np.float32)
        h0[:len(idx0[p])] = hc[idx0[p]]
        h1 = np.zeros((NB1, D), np.float32)
        h1[:len(idx1[p])] = hc[idx1[p]]
        acts = {
            "hT": bf(_interleave_k(np.ascontiguousarray(hc.T), KT)),
            "h0T": bf(_interleave_k(np.ascontiguousarray(h0.T), KT)),
            "h1T": bf(_interleave_k(np.ascontiguousarray(h1.T), KT)),
        }
        for j in range(2):
            in_maps.append({**acts, **w_common, **halves[j]})

    profile = bool(os.environ.get("BASS_KERNEL_PROFILE"))
    if profile:
        try:
            _install_ntff_hook()
        except Exception:
            pass
    res = _run_spmd(nc, in_maps, profile)
    if profile and res.exec_time_ns is not None:
        print(f"HW exec time: {res.exec_time_ns} ns")
        kernel.last_exec_time_ns = res.exec_time_ns

    head_out = np.empty((N, VH), np.float32)
    tail0 = np.zeros((N, VT), np.float32)
    tail1 = np.zeros((N, VT), np.float32)
    for p in range(N_PAIRS):
        base = p * TOK
        for j in range(2):
            r = res.results[2 * p + j]
            head_out[base:base + TOK, j * VHH:(j + 1) * VHH] = \
                r["head_o"].astype(np.float32)
            if len(idx0[p]):
                tail0[base + idx0[p], j * VTH:(j + 1) * VTH] = \
                    r["t0_o"][:len(idx0[p])].astype(np.float32)
            if len(idx1[p]):
                tail1[base + idx1[p], j * VTH:(j + 1) * VTH] = \
                    r["t1_o"][:len(idx1[p])].astype(np.float32)

    return (head_out, tail0, tail1, new_t_head, new_t0, new_t1)


# revision 36
# speedup vs baseline: 1.2125x; 1.0044x over previous
"""AdaptiveSoftmax forward on 8 TRN2 NeuronCores.

Strategy: data-parallel over token pairs x 2-way tensor-parallel over
vocab, no collectives.
 - cores (2p, 2p+1) share the same 1024 tokens; the even core computes
   the left vocab half, the odd core the right half (head 5001/5001,
   tails 10000/10000) -> identical SPMD graph, different input data.
 - tails: moe-style routing — band rows are compacted on host, each core
   computes proj+out matmuls only for the pair's ~410 band tokens
   (padded to a static NB), and the host scatters results into the
   zero-filled dense output (out-of-band reference rows are exactly 0).
 - all device matmuls in bf16 (inputs pre-transposed/cast/interleaved on
   host so every SBUF strip is one or two large contiguous DMAs), fp32
   PSUM accumulation, bf16 outputs upcast on host.
 - head/tail0/tail1 column groups are interleaved in emission order so
   the output-DMA rate stays under the HBM roofline while the
   TensorEngine stays busy.
 - the tiny int32 retarget vectors are pure indexing; computed on host.
"""

import os

import numpy as np
import ml_dtypes

CUTOFF = (10000, 30000, 50000)
D = 1024
VH = CUTOFF[0] + 2          # 10002
VT = CUTOFF[1] - CUTOFF[0]  # 20000 (both tails)
VHH = VH // 2               # 5001 per-core head half
VTH = VT // 2               # 10000 per-core tail half
D0 = 256
D1 = 64
N_CORES = 8
N_PAIRS = N_CORES // 2
P = 128
KT = D // P                 # 8 contraction tiles over D

BF16 = ml_dtypes.bfloat16

_graph_cache = {}


def _groups(total, tile_w, group_tiles):
    """[(col0, [tile widths]), ...] covering `total` columns."""
    tiles, c = [], 0
    while c < total:
        tiles.append(min(tile_w, total - c))
        c += tiles[-1]
    out, c = [], 0
    for i in range(0, len(tiles), group_tiles):
        chunk = tiles[i:i + group_tiles]
        out.append((c, chunk))
        c += sum(chunk)
    return out


HEAD_GROUPS = _groups(VHH, 512, 2)   # 5 groups: 4x1024 + 905
TAIL_GROUPS = _groups(VTH, 512, 4)   # 5 groups: 4x2048 + 1808


def _build_graph(TOK, NB0, NB1):
    """SPMD bacc graph. TOK tokens per pair; NB0/NB1 padded band sizes."""
    import concourse.bacc as bacc
    import concourse.tile as tile
    import concourse.mybir as mybir

    f32 = mybir.dt.float32
    bf16 = mybir.dt.bfloat16

    MT = TOK // P
    M0 = NB0 // P
    M1 = NB1 // P

    nc = bacc.Bacc("TRN2", target_bir_lowering=False, debug=False,
                   num_devices=N_CORES)

    hT_e = nc.dram_tensor("hT", (P, KT * TOK), bf16, kind="ExternalInput")
    h0T_e = nc.dram_tensor("h0T", (P, KT * NB0), bf16, kind="ExternalInput")
    h1T_e = nc.dram_tensor("h1T", (P, KT * NB1), bf16, kind="ExternalInput")
    hw_e = nc.dram_tensor("head_wI", (P, KT * VHH), bf16, kind="ExternalInput")
    p0w_e = nc.dram_tensor("proj0_wI", (P, KT * D0), bf16, kind="ExternalInput")
    p1w_e = nc.dram_tensor("proj1_wI", (P, KT * D1), bf16, kind="ExternalInput")
    o0w_e = nc.dram_tensor("out0_wI", (P, 2 * VTH), bf16, kind="ExternalInput")
    # out1 rows duplicated into both partition halves for row-packed matmuls
    o1w_e = nc.dram_tensor("out1_wD", (P, VTH), bf16, kind="ExternalInput")
    ho_e = nc.dram_tensor("head_o", (TOK, VHH), bf16, kind="ExternalOutput")
    t0_e = nc.dram_tensor("t0_o", (NB0, VTH), bf16, kind="ExternalOutput")
    t1_e = nc.dram_tensor("t1_o", (NB1, VTH), bf16, kind="ExternalOutput")

    copy_ctr = [0]

    with tile.TileContext(nc) as tc:
        with (
            tc.tile_pool(name="res", bufs=1) as rpool,
            tc.tile_pool(name="wstrip", bufs=5) as wpool,
            tc.tile_pool(name="ostage", bufs=10) as opool,
            tc.tile_pool(name="psmain", bufs=6, space="PSUM") as psm,
            tc.tile_pool(name="psproj", bufs=1, space="PSUM") as psp,
        ):
            def evict(dst, src):
                if copy_ctr[0] % 2 == 0:
                    nc.scalar.copy(dst, src)
                else:
                    nc.vector.tensor_copy(dst, src)
                copy_ctr[0] += 1

            def dma_split(dst, src, n):
                F = dst.shape[-1]
                step = -(-F // n)
                j = 0
                while j < F:
                    w = min(step, F - j)
                    nc.sync.dma_start(dst[:, j:j + w], src[:, j:j + w])
                    j += w

            # ---- PE warm-up: dummy matmuls on a zeroed tile keep the
            # TensorEngine active (HAM un-throttled) while the first
            # input DMAs stream in; results are never read ----
            warm = rpool.tile([P, 512], bf16, tag="warm")
            nc.vector.memset(warm[:], 0.0)
            wps = psm.tile([P, 512], f32, tag="ps", name="warm_ps")
            N_WARM = 40
            for i in range(N_WARM):
                nc.tensor.matmul(warm_out := wps[:], warm[:, 0:P], warm[:],
                                 start=(i == 0), stop=(i == N_WARM - 1))

            # ---- resident loads; proj inputs first so proj can start ----
            p0w = rpool.tile([P, KT * D0], bf16, tag="p0w")
            nc.sync.dma_start(p0w[:], p0w_e.ap()[:])
            h0T = rpool.tile([P, KT * NB0], bf16, tag="h0T")
            dma_split(h0T[:], h0T_e.ap()[:], 2)
            p1w = rpool.tile([P, KT * D1], bf16, tag="p1w")
            nc.sync.dma_start(p1w[:], p1w_e.ap()[:])
            h1T = rpool.tile([P, KT * NB1], bf16, tag="h1T")
            dma_split(h1T[:], h1T_e.ap()[:], 2)
            hT = rpool.tile([P, KT * TOK], bf16, tag="hT")
            dma_split(hT[:], hT_e.ap()[:], 4)

            # ---- proj0: p0T[d0, NB0] (2 partition tiles) ----
            p0T = rpool.tile([P, 2 * NB0], bf16, tag="p0T")
            for mp in range(2):
                ps = psp.tile([P, NB0], f32, tag="ps0")
                for k in range(KT):
                    nc.tensor.matmul(
                        ps[:],
                        p0w[:, k * D0 + mp * P: k * D0 + (mp + 1) * P],
                        h0T[:, k * NB0:(k + 1) * NB0],
                        start=(k == 0), stop=(k == KT - 1),
                    )
                evict(p0T[:, mp * NB0:(mp + 1) * NB0], ps[:])

            # ---- proj1: p1T[d1, NB1], duplicated into both partition halves
            p1T = rpool.tile([P, NB1], bf16, tag="p1T")
            ps = psp.tile([D1, NB1], f32, tag="ps1")
            for k in range(KT):
                nc.tensor.matmul(
                    ps[:],
                    p1w[:, k * D1:(k + 1) * D1],
                    h1T[:, k * NB1:(k + 1) * NB1],
                    start=(k == 0), stop=(k == KT - 1),
                )
            evict(p1T[0:D1, :], ps[:])
            evict(p1T[D1:P, :], ps[:])

            # ---- one column group of an output matmul ----
            def out_group(c0, tiles, KP, strip_part, w_e, w_il, out_e, M,
                          lhsT_fn, strip_splits, stag="ws"):
                W = sum(tiles)
                strip = wpool.tile([strip_part, KP * W], bf16, tag=stag,
                                   name=f"strip_{stag}_{c0}",
                                   bufs=3 if stag == "wsh" else 4)
                if w_il:
                    dma_split(strip[:], w_e.ap()[:, KP * c0: KP * (c0 + W)],
                              strip_splits)
                else:
                    dma_split(strip[:], w_e.ap()[:, c0:c0 + W], strip_splits)
                for m in range(M):
                    ot = opool.tile([P, W], bf16, tag="o")
                    j = 0
                    for nsz in tiles:
                        ps = psm.tile([P, nsz], f32, tag="ps")
                        for kp in range(KP):
                            nc.tensor.matmul(
                                ps[:],
                                lhsT_fn(kp, m),
                                strip[:, kp * W + j: kp * W + j + nsz],
                                start=(kp == 0), stop=(kp == KP - 1),
                            )
                        evict(ot[:, j:j + nsz], ps[:])
                        j += nsz
                    dma_split(out_e.ap()[m * P:(m + 1) * P, c0:c0 + W],
                              ot[:], 2)

            # ---- tail1 group: K=64, two m-tiles packed into the array ----
            def t1_group(c0, tiles):
                W = sum(tiles)
                strip = wpool.tile([P, W], bf16, tag="ws",
                                   name=f"strip_t1_{c0}", bufs=4)
                dma_split(strip[:], o1w_e.ap()[:, c0:c0 + W], 2)
                for mp in range(0, M1 - (M1 % 2), 2):
                    ot_a = opool.tile([P, W], bf16, tag="o", name=f"ot1a_{c0}_{mp}")
                    ot_b = opool.tile([P, W], bf16, tag="o", name=f"ot1b_{c0}_{mp}")
                    j = 0
                    for nsz in tiles:
                        ps_a = psm.tile([P, nsz], f32, tag="ps", name=f"pt1a_{c0}_{mp}_{j}")
                        ps_b = psm.tile([P, nsz], f32, tag="ps", name=f"pt1b_{c0}_{mp}_{j}")
                        nc.tensor.matmul(
                            ps_a[:], p1T[0:D1, mp * P:(mp + 1) * P],
                            strip[0:D1, j:j + nsz], start=True, stop=True)
                        nc.tensor.matmul(
                            ps_b[:], p1T[D1:P, (mp + 1) * P:(mp + 2) * P],
                            strip[D1:P, j:j + nsz], start=True, stop=True)
                        evict(ot_a[:, j:j + nsz], ps_a[:])
                        evict(ot_b[:, j:j + nsz], ps_b[:])
                        j += nsz
                    dma_split(t1_e.ap()[mp * P:(mp + 1) * P, c0:c0 + W],
                              ot_a[:], 2)
                    dma_split(t1_e.ap()[(mp + 1) * P:(mp + 2) * P, c0:c0 + W],
                              ot_b[:], 2)
                if M1 % 2:
                    m = M1 - 1
                    ot = opool.tile([P, W], bf16, tag="o", name=f"ot1c_{c0}")
                    j = 0
                    for nsz in tiles:
                        ps = psm.tile([P, nsz], f32, tag="ps", name=f"pt1c_{c0}_{j}")
                        nc.tensor.matmul(
                            ps[:], p1T[0:D1, m * P:(m + 1) * P],
                            strip[0:D1, j:j + nsz], start=True, stop=True)
                        evict(ot[:, j:j + nsz], ps[:])
                        j += nsz
                    dma_split(t1_e.ap()[m * P:(m + 1) * P, c0:c0 + W],
                              ot[:], 2)

            # ---- interleave column groups: head leads each round (its
            # strips are prefetched and it is PE-dense, absorbing the
            # DMA bursts of the tail groups that follow) ----
            def head_group(i):
                c0, tiles = HEAD_GROUPS[i]
                out_group(
                    c0, tiles, KT, P, hw_e, True, ho_e, MT,
                    lambda k, m: hT[:, k * TOK + m * P: k * TOK + (m + 1) * P],
                    4, stag="wsh")

            def tail_groups(i):
                c0, tiles = TAIL_GROUPS[i]
                t1_group(c0, tiles)
                out_group(
                    c0, tiles, 2, P, o0w_e, True, t0_e, M0,
                    lambda k, m: p0T[:, k * NB0 + m * P: k * NB0 + (m + 1) * P],
                    4)

            # H0 T0 H1 T1 H2 T2 H3 T3 T4 H4 — starts and ends PE-dense so
            # the tail groups' output-DMA bursts overlap head compute.
            nH, nT = len(HEAD_GROUPS), len(TAIL_GROUPS)
            head_group(0)
            for i in range(1, nH - 1):
                tail_groups(i - 1)
                head_group(i)
            for i in range(nH - 2, nT):
                tail_groups(i)
            head_group(nH - 1)

    nc.compile()
    return nc


def _pad128(n):
    return max(P, ((n + P - 1) // P) * P)


def _install_ntff_hook():
    """Register the NTFF profile hook the agent image's antenv lacks, so
    run_bass_kernel_spmd(trace=True) can report exec_time_ns under axon."""
    import sys
    import types
    if 'antenv.axon_hooks' in sys.modules:
        return
    mod = types.ModuleType('antenv.axon_hooks')
    mod._hook = None
    mod.set_axon_ntff_profile_hook = lambda h: setattr(mod, '_hook', h)
    mod.get_axon_ntff_profile_hook = lambda: mod._hook
    sys.modules['antenv.axon_hooks'] = mod
    import antenv
    antenv.axon_hooks = mod
    from trn_agent_boot.trn_boot import _ntff_profile_via_ctypes
    mod._hook = _ntff_profile_via_ctypes('/opt/axon/libaxon_pjrt.so')
    import concourse.bass_utils as bu
    bu.upload_artifacts = lambda tmpdir: f"local:{tmpdir}"


def _run_spmd(nc, in_maps, profile):
    from concourse.bass_utils import run_bass_kernel_spmd
    kwargs = {}
    if profile:
        kwargs["trace"] = True
    return run_bass_kernel_spmd(nc, in_maps, core_ids=list(range(N_CORES)),
                                **kwargs)


def _interleave_k(a_t, kt):
    """[kt*P, F] -> [P, kt*F] with X[p, k*F + j] = a_t[k*P + p, j]."""
    ktp, F = a_t.shape
    assert ktp == kt * P
    return np.ascontiguousarray(
        a_t.reshape(kt, P, F).transpose(1, 0, 2).reshape(P, kt * F))


def _interleave_groups(a_t, kt, groups):
    """Group-blocked interleave: for each (c0, tiles) with width W, block
    [P, kt*W] with X[p, k*W + j] = a_t[k*P + p, c0 + j]; blocks concat."""
    A = a_t.reshape(kt, P, a_t.shape[1])
    blocks = []
    for c0, tiles in groups:
        W = sum(tiles)
        blocks.append(A[:, :, c0:c0 + W].transpose(1, 0, 2).reshape(P, kt * W))
    return np.ascontiguousarray(np.concatenate(blocks, axis=1))


def kernel(input, target, head_w, proj0_w, out0_w, proj1_w, out1_w):
    c0_, c1_, c2_ = CUTOFF
    h = np.ascontiguousarray(np.asarray(input, dtype=np.float32)).reshape(-1, D)
    t = np.asarray(target, dtype=np.int32).reshape(-1)
    N = h.shape[0]
    assert N % N_CORES == 0
    TOK = N // N_PAIRS          # tokens per core pair

    m0 = (t >= c0_) & (t < c1_)
    m1 = (t >= c1_) & (t < c2_)

    # int32 retarget outputs (pure indexing, negligible)
    new_t_head = np.where(m0, c0_, np.where(m1, c0_ + 1, t)).astype(np.int32)
    new_t0 = np.where(m0, t - c0_, 0).astype(np.int32)
    new_t1 = np.where(m1, t - c1_, 0).astype(np.int32)

    # per-pair routing of band rows
    idx0 = [np.nonzero(m0[p * TOK:(p + 1) * TOK])[0] for p in range(N_PAIRS)]
    idx1 = [np.nonzero(m1[p * TOK:(p + 1) * TOK])[0] for p in range(N_PAIRS)]
    NB0 = _pad128(max(len(i) for i in idx0))
    NB1 = _pad128(max(len(i) for i in idx1))

    key = (TOK, NB0, NB1)
    if key not in _graph_cache:
        _graph_cache[key] = _build_graph(TOK, NB0, NB1)
    nc = _graph_cache[key]

    bf = lambda a: a.astype(BF16)

    hw_t = np.asarray(head_w, np.float32).T      # [D, VH]
    o0_t = np.asarray(out0_w, np.float32).T      # [D0, VT]
    o1_t = np.asarray(out1_w, np.float32).T      # [D1, VT]
    halves = []
    for j in range(2):
        halves.append({
            "head_wI": bf(_interleave_groups(
                np.ascontiguousarray(hw_t[:, j * VHH:(j + 1) * VHH]),
                KT, HEAD_GROUPS)),
            "out0_wI": bf(_interleave_groups(
                np.ascontiguousarray(o0_t[:, j * VTH:(j + 1) * VTH]),
                2, TAIL_GROUPS)),
            "out1_wD": bf(np.ascontiguousarray(np.concatenate(
                [o1_t[:, j * VTH:(j + 1) * VTH]] * 2, axis=0))),
        })
    w_common = {
        "proj0_wI": bf(_interleave_k(np.ascontiguousarray(
            np.asarray(proj0_w, np.float32).T), KT)),
        "proj1_wI": bf(_interleave_k(np.ascontiguousarray(
            np.asarray(proj1_w, np.float32).T), KT)),
    }

    in_maps = []
    for p in range(N_PAIRS):
        hc = h[p * TOK:(p + 1) * TOK]
        h0 = np.zeros((NB0, D), np.float32)
        h0[:len(idx0[p])] = hc[idx0[p]]
        h1 = np.zeros((NB1, D), np.float32)
        h1[:len(idx1[p])] = hc[idx1[p]]
        acts = {
            "hT": bf(_interleave_k(np.ascontiguousarray(hc.T), KT)),
            "h0T": bf(_interleave_k(np.ascontiguousarray(h0.T), KT)),
            "h1T": bf(_interleave_k(np.ascontiguousarray(h1.T), KT)),
        }
        for j in range(2):
            in_maps.append({**acts, **w_common, **halves[j]})

    profile = bool(os.environ.get("BASS_KERNEL_PROFILE"))
    if profile:
        try:
            _install_ntff_hook()
        except Exception:
            pass
    res = _run_spmd(nc, in_maps, profile)
    if profile and res.exec_time_ns is not None:
        print(f"HW exec time: {res.exec_time_ns} ns")
        kernel.last_exec_time_ns = res.exec_time_ns

    head_out = np.empty((N, VH), np.float32)
    tail0 = np.zeros((N, VT), np.float32)
    tail1 = np.zeros((N, VT), np.float32)
    for p in range(N_PAIRS):
        base = p * TOK
        for j in range(2):
            r = res.results[2 * p + j]
            head_out[base:base + TOK, j * VHH:(j + 1) * VHH] = \
                r["head_o"].astype(np.float32)
            if len(idx0[p]):
                tail0[base + idx0[p], j * VTH:(j + 1) * VTH] = \
                    r["t0_o"][:len(idx0[p])].astype(np.float32)
            if len(idx1[p]):
                tail1[base + idx1[p], j * VTH:(j + 1) * VTH] = \
                    r["t1_o"][:len(idx1[p])].astype(np.float32)

    return (head_out, tail0, tail1, new_t_head, new_t0, new_t1)


# revision 41
# speedup vs baseline: 1.2233x; 1.0089x over previous
"""AdaptiveSoftmax forward on 8 TRN2 NeuronCores.

Strategy: data-parallel over token pairs x 2-way tensor-parallel over
vocab, no collectives.
 - cores (2p, 2p+1) share the same 1024 tokens; the even core computes
   the left vocab half, the odd core the right half (head 5001/5001,
   tails 10000/10000) -> identical SPMD graph, different input data.
 - tails: moe-style routing — band rows are compacted on host, each core
   computes proj+out matmuls only for the pair's ~410 band tokens
   (padded to a static NB), and the host scatters results into the
   zero-filled dense output (out-of-band reference rows are exactly 0).
 - all device matmuls in bf16 (inputs pre-transposed/cast/interleaved on
   host so every SBUF strip is one or two large contiguous DMAs), fp32
   PSUM accumulation, bf16 outputs upcast on host.
 - head/tail0/tail1 column groups are interleaved in emission order so
   the output-DMA rate stays under the HBM roofline while the
   TensorEngine stays busy.
 - the tiny int32 retarget vectors are pure indexing; computed on host.
"""

import os

import numpy as np
import ml_dtypes

CUTOFF = (10000, 30000, 50000)
D = 1024
VH = CUTOFF[0] + 2          # 10002
VT = CUTOFF[1] - CUTOFF[0]  # 20000 (both tails)
VHH = VH // 2               # 5001 per-core head half
VTH = VT // 2               # 10000 per-core tail half
D0 = 256
D1 = 64
N_CORES = 8
N_PAIRS = N_CORES // 2
P = 128
KT = D // P                 # 8 contraction tiles over D

BF16 = ml_dtypes.bfloat16

_graph_cache = {}


def _groups(total, tile_w, group_tiles):
    """[(col0, [tile widths]), ...] covering `total` columns."""
    tiles, c = [], 0
    while c < total:
        tiles.append(min(tile_w, total - c))
        c += tiles[-1]
    out, c = [], 0
    for i in range(0, len(tiles), group_tiles):
        chunk = tiles[i:i + group_tiles]
        out.append((c, chunk))
        c += sum(chunk)
    return out


HEAD_GROUPS = _groups(VHH, 512, 2)   # 5 groups: 4x1024 + 905
TAIL_GROUPS = _groups(VTH, 512, 4)   # 5 groups: 4x2048 + 1808


def _build_graph(TOK, NB0, NB1):
    """SPMD bacc graph. TOK tokens per pair; NB0/NB1 padded band sizes."""
    import concourse.bacc as bacc
    import concourse.tile as tile
    import concourse.mybir as mybir

    f32 = mybir.dt.float32
    bf16 = mybir.dt.bfloat16

    MT = TOK // P
    M0 = NB0 // P
    M1 = NB1 // P

    nc = bacc.Bacc("TRN2", target_bir_lowering=False, debug=False,
                   num_devices=N_CORES)

    hT_e = nc.dram_tensor("hT", (P, KT * TOK), bf16, kind="ExternalInput")
    h0T_e = nc.dram_tensor("h0T", (P, KT * NB0), bf16, kind="ExternalInput")
    h1T_e = nc.dram_tensor("h1T", (P, KT * NB1), bf16, kind="ExternalInput")
    hw_e = nc.dram_tensor("head_wI", (P, KT * VHH), bf16, kind="ExternalInput")
    p0w_e = nc.dram_tensor("proj0_wI", (P, KT * D0), bf16, kind="ExternalInput")
    p1w_e = nc.dram_tensor("proj1_wI", (P, KT * D1), bf16, kind="ExternalInput")
    o0w_e = nc.dram_tensor("out0_wI", (P, 2 * VTH), bf16, kind="ExternalInput")
    # out1 rows duplicated into both partition halves for row-packed matmuls
    o1w_e = nc.dram_tensor("out1_wD", (P, VTH), bf16, kind="ExternalInput")
    ho_e = nc.dram_tensor("head_o", (TOK, VHH), bf16, kind="ExternalOutput")
    t0_e = nc.dram_tensor("t0_o", (NB0, VTH), bf16, kind="ExternalOutput")
    t1_e = nc.dram_tensor("t1_o", (NB1, VTH), bf16, kind="ExternalOutput")

    copy_ctr = [0]

    with tile.TileContext(nc) as tc:
        with (
            tc.tile_pool(name="res", bufs=1) as rpool,
            tc.tile_pool(name="wstrip", bufs=5) as wpool,
            tc.tile_pool(name="ostage", bufs=10) as opool,
            tc.tile_pool(name="psmain", bufs=6, space="PSUM") as psm,
            tc.tile_pool(name="psproj", bufs=1, space="PSUM") as psp,
        ):
            def evict(dst, src):
                if copy_ctr[0] % 2 == 0:
                    nc.scalar.copy(dst, src)
                else:
                    nc.vector.tensor_copy(dst, src)
                copy_ctr[0] += 1

            def dma_split(dst, src, n):
                F = dst.shape[-1]
                step = -(-F // n)
                j = 0
                while j < F:
                    w = min(step, F - j)
                    nc.sync.dma_start(dst[:, j:j + w], src[:, j:j + w])
                    j += w

            # ---- PE warm-up: dummy matmuls on a zeroed tile keep the
            # TensorEngine active (HAM un-throttled) while the first
            # input DMAs stream in; results are never read ----
            warm = rpool.tile([P, 512], bf16, tag="warm")
            nc.vector.memset(warm[:], 0.0)
            wps = psm.tile([P, 512], f32, tag="ps", name="warm_ps")
            N_WARM = 40
            for i in range(N_WARM):
                nc.tensor.matmul(warm_out := wps[:], warm[:, 0:P], warm[:],
                                 start=(i == 0), stop=(i == N_WARM - 1))

            # ---- resident loads; proj inputs first so proj can start ----
            p0w = rpool.tile([P, KT * D0], bf16, tag="p0w")
            nc.sync.dma_start(p0w[:], p0w_e.ap()[:])
            h0T = rpool.tile([P, KT * NB0], bf16, tag="h0T")
            dma_split(h0T[:], h0T_e.ap()[:], 2)
            p1w = rpool.tile([P, KT * D1], bf16, tag="p1w")
            nc.sync.dma_start(p1w[:], p1w_e.ap()[:])
            h1T = rpool.tile([P, KT * NB1], bf16, tag="h1T")
            dma_split(h1T[:], h1T_e.ap()[:], 2)
            hT = rpool.tile([P, KT * TOK], bf16, tag="hT")
            dma_split(hT[:], hT_e.ap()[:], 4)

            # ---- proj0: p0T[d0, NB0] (2 partition tiles) ----
            p0T = rpool.tile([P, 2 * NB0], bf16, tag="p0T")
            for mp in range(2):
                ps = psp.tile([P, NB0], f32, tag="ps0")
                for k in range(KT):
                    nc.tensor.matmul(
                        ps[:],
                        p0w[:, k * D0 + mp * P: k * D0 + (mp + 1) * P],
                        h0T[:, k * NB0:(k + 1) * NB0],
                        start=(k == 0), stop=(k == KT - 1),
                    )
                evict(p0T[:, mp * NB0:(mp + 1) * NB0], ps[:])

            # ---- proj1: p1T[d1, NB1], duplicated into both partition halves
            p1T = rpool.tile([P, NB1], bf16, tag="p1T")
            ps = psp.tile([D1, NB1], f32, tag="ps1")
            for k in range(KT):
                nc.tensor.matmul(
                    ps[:],
                    p1w[:, k * D1:(k + 1) * D1],
                    h1T[:, k * NB1:(k + 1) * NB1],
                    start=(k == 0), stop=(k == KT - 1),
                )
            evict(p1T[0:D1, :], ps[:])
            evict(p1T[D1:P, :], ps[:])

            # ---- one column group of an output matmul ----
            def out_group(c0, tiles, KP, strip_part, w_e, w_il, out_e, M,
                          lhsT_fn, strip_splits, stag="ws"):
                W = sum(tiles)
                strip = wpool.tile([strip_part, KP * W], bf16, tag=stag,
                                   name=f"strip_{stag}_{c0}",
                                   bufs=3 if stag == "wsh" else 4)
                if w_il:
                    dma_split(strip[:], w_e.ap()[:, KP * c0: KP * (c0 + W)],
                              strip_splits)
                else:
                    dma_split(strip[:], w_e.ap()[:, c0:c0 + W], strip_splits)
                for m in range(M):
                    ot = opool.tile([P, W], bf16, tag="o")
                    j = 0
                    for nsz in tiles:
                        ps = psm.tile([P, nsz], f32, tag="ps")
                        for kp in range(KP):
                            nc.tensor.matmul(
                                ps[:],
                                lhsT_fn(kp, m),
                                strip[:, kp * W + j: kp * W + j + nsz],
                                start=(kp == 0), stop=(kp == KP - 1),
                            )
                        evict(ot[:, j:j + nsz], ps[:])
                        j += nsz
                    dma_split(out_e.ap()[m * P:(m + 1) * P, c0:c0 + W],
                              ot[:], 2)

            # ---- tail1 group: K=64, two m-tiles packed into the array ----
            def t1_group(c0, tiles):
                W = sum(tiles)
                strip = wpool.tile([P, W], bf16, tag="ws",
                                   name=f"strip_t1_{c0}", bufs=4)
                dma_split(strip[:], o1w_e.ap()[:, c0:c0 + W], 2)
                for mp in range(0, M1 - (M1 % 2), 2):
                    ot_a = opool.tile([P, W], bf16, tag="o", name=f"ot1a_{c0}_{mp}")
                    ot_b = opool.tile([P, W], bf16, tag="o", name=f"ot1b_{c0}_{mp}")
                    j = 0
                    for nsz in tiles:
                        ps_a = psm.tile([P, nsz], f32, tag="ps", name=f"pt1a_{c0}_{mp}_{j}")
                        ps_b = psm.tile([P, nsz], f32, tag="ps", name=f"pt1b_{c0}_{mp}_{j}")
                        nc.tensor.matmul(
                            ps_a[:], p1T[0:D1, mp * P:(mp + 1) * P],
                            strip[0:D1, j:j + nsz], start=True, stop=True)
                        nc.tensor.matmul(
                            ps_b[:], p1T[D1:P, (mp + 1) * P:(mp + 2) * P],
                            strip[D1:P, j:j + nsz], start=True, stop=True)
                        evict(ot_a[:, j:j + nsz], ps_a[:])
                        evict(ot_b[:, j:j + nsz], ps_b[:])
                        j += nsz
                    dma_split(t1_e.ap()[mp * P:(mp + 1) * P, c0:c0 + W],
                              ot_a[:], 2)
                    dma_split(t1_e.ap()[(mp + 1) * P:(mp + 2) * P, c0:c0 + W],
                              ot_b[:], 2)
                if M1 % 2:
                    m = M1 - 1
                    ot = opool.tile([P, W], bf16, tag="o", name=f"ot1c_{c0}")
                    j = 0
                    for nsz in tiles:
                        ps = psm.tile([P, nsz], f32, tag="ps", name=f"pt1c_{c0}_{j}")
                        nc.tensor.matmul(
                            ps[:], p1T[0:D1, m * P:(m + 1) * P],
                            strip[0:D1, j:j + nsz], start=True, stop=True)
                        evict(ot[:, j:j + nsz], ps[:])
                        j += nsz
                    dma_split(t1_e.ap()[m * P:(m + 1) * P, c0:c0 + W],
                              ot[:], 2)

            # ---- interleave column groups: head leads each round (its
            # strips are prefetched and it is PE-dense, absorbing the
            # DMA bursts of the tail groups that follow) ----
            def head_group(i):
                c0, tiles = HEAD_GROUPS[i]
                out_group(
                    c0, tiles, KT, P, hw_e, True, ho_e, MT,
                    lambda k, m: hT[:, k * TOK + m * P: k * TOK + (m + 1) * P],
                    4, stag="wsh")

            def tail_groups(i):
                c0, tiles = TAIL_GROUPS[i]
                t1_group(c0, tiles)
                out_group(
                    c0, tiles, 2, P, o0w_e, True, t0_e, M0,
                    lambda k, m: p0T[:, k * NB0 + m * P: k * NB0 + (m + 1) * P],
                    4)

            # H0 T0 H1 T1 H2 T2 H3 T3 T4 H4 — starts and ends PE-dense so
            # the tail groups' output-DMA bursts overlap head compute.
            nH, nT = len(HEAD_GROUPS), len(TAIL_GROUPS)
            head_group(0)
            for i in range(1, nH - 1):
                tail_groups(i - 1)
                head_group(i)
            for i in range(nH - 2, nT):
                tail_groups(i)
            head_group(nH - 1)

    nc.compile()
    return nc


def _pad128(n):
    return max(P, ((n + P - 1) // P) * P)


def _install_ntff_hook():
    """Register the NTFF profile hook the agent image's antenv lacks, so
    run_bass_kernel_spmd(trace=True) can report exec_time_ns under axon."""
    import sys
    import types
    if 'antenv.axon_hooks' in sys.modules:
        return
    mod = types.ModuleType('antenv.axon_hooks')
    mod._hook = None
    mod.set_axon_ntff_profile_hook = lambda h: setattr(mod, '_hook', h)
    mod.get_axon_ntff_profile_hook = lambda: mod._hook
    sys.modules['antenv.axon_hooks'] = mod
    import antenv
    antenv.axon_hooks = mod
    from trn_agent_boot.trn_boot import _ntff_profile_via_ctypes
    mod._hook = _ntff_profile_via_ctypes('/opt/axon/libaxon_pjrt.so')
    import concourse.bass_utils as bu
    bu.upload_artifacts = lambda tmpdir: f"local:{tmpdir}"


def _run_spmd(nc, in_maps, profile):
    from concourse.bass_utils import run_bass_kernel_spmd
    kwargs = {}
    if profile:
        kwargs["trace"] = True
    return run_bass_kernel_spmd(nc, in_maps, core_ids=list(range(N_CORES)),
                                **kwargs)


def _interleave_k(a_t, kt):
    """[kt*P, F] -> [P, kt*F] with X[p, k*F + j] = a_t[k*P + p, j]."""
    ktp, F = a_t.shape
    assert ktp == kt * P
    return np.ascontiguousarray(
        a_t.reshape(kt, P, F).transpose(1, 0, 2).reshape(P, kt * F))


def _interleave_groups(a_t, kt, groups):
    """Group-blocked interleave: for each (c0, tiles) with width W, block
    [P, kt*W] with X[p, k*W + j] = a_t[k*P + p, c0 + j]; blocks concat."""
    A = a_t.reshape(kt, P, a_t.shape[1])
    blocks = []
    for c0, tiles in groups:
        W = sum(tiles)
        blocks.append(A[:, :, c0:c0 + W].transpose(1, 0, 2).reshape(P, kt * W))
    return np.ascontiguousarray(np.concatenate(blocks, axis=1))


def kernel(input, target, head_w, proj0_w, out0_w, proj1_w, out1_w):
    c0_, c1_, c2_ = CUTOFF
    h = np.ascontiguousarray(np.asarray(input, dtype=np.float32)).reshape(-1, D)
    t = np.asarray(target, dtype=np.int32).reshape(-1)
    N = h.shape[0]
    assert N % N_CORES == 0
    TOK = N // N_PAIRS          # tokens per core pair

    m0 = (t >= c0_) & (t < c1_)
    m1 = (t >= c1_) & (t < c2_)

    # int32 retarget outputs (pure indexing, negligible)
    new_t_head = np.where(m0, c0_, np.where(m1, c0_ + 1, t)).astype(np.int32)
    new_t0 = np.where(m0, t - c0_, 0).astype(np.int32)
    new_t1 = np.where(m1, t - c1_, 0).astype(np.int32)

    # per-pair routing of band rows
    idx0 = [np.nonzero(m0[p * TOK:(p + 1) * TOK])[0] for p in range(N_PAIRS)]
    idx1 = [np.nonzero(m1[p * TOK:(p + 1) * TOK])[0] for p in range(N_PAIRS)]
    NB0 = _pad128(max(len(i) for i in idx0))
    NB1 = _pad128(max(len(i) for i in idx1))

    key = (TOK, NB0, NB1)
    if key not in _graph_cache:
        _graph_cache[key] = _build_graph(TOK, NB0, NB1)
    nc = _graph_cache[key]

    bf = lambda a: a.astype(BF16)

    hw_t = np.asarray(head_w, np.float32).T      # [D, VH]
    o0_t = np.asarray(out0_w, np.float32).T      # [D0, VT]
    o1_t = np.asarray(out1_w, np.float32).T      # [D1, VT]
    halves = []
    for j in range(2):
        halves.append({
            "head_wI": bf(_interleave_groups(
                np.ascontiguousarray(hw_t[:, j * VHH:(j + 1) * VHH]),
                KT, HEAD_GROUPS)),
            "out0_wI": bf(_interleave_groups(
                np.ascontiguousarray(o0_t[:, j * VTH:(j + 1) * VTH]),
                2, TAIL_GROUPS)),
            "out1_wD": bf(np.ascontiguousarray(np.concatenate(
                [o1_t[:, j * VTH:(j + 1) * VTH]] * 2, axis=0))),
        })
    w_common = {
        "proj0_wI": bf(_interleave_k(np.ascontiguousarray(
            np.asarray(proj0_w, np.float32).T), KT)),
        "proj1_wI": bf(_interleave_k(np.ascontiguousarray(
            np.asarray(proj1_w, np.float32).T), KT)),
    }

    in_maps = []
    for p in range(N_PAIRS):
        hc = h[p * TOK:(p + 1) * TOK]
        h0 = np.zeros((NB0, D), np.float32)
        h0[:len(idx0[p])] = hc[idx0[p]]
        h1 = np.zeros((NB1, D), np.float32)
        h1[:len(idx1[p])] = hc[idx1[p]]
        acts = {
            "hT": bf(_interleave_k(np.ascontiguousarray(hc.T), KT)),
            "h0T": bf(_interleave_k(np.ascontiguousarray(h0.T), KT)),
            "h1T": bf(_interleave_k(np.ascontiguousarray(h1.T), KT)),
        }
        for j in range(2):
            in_maps.append({**acts, **w_common, **halves[j]})

    profile = bool(os.environ.get("BASS_KERNEL_PROFILE"))
    if profile:
        try:
            _install_ntff_hook()
        except Exception:
            pass
    res = _run_spmd(nc, in_maps, profile)
    if profile and res.exec_time_ns is not None:
        print(f"HW exec time: {res.exec_time_ns} ns")
        kernel.last_exec_time_ns = res.exec_time_ns

    head_out = np.empty((N, VH), np.float32)
    tail0 = np.zeros((N, VT), np.float32)
    tail1 = np.zeros((N, VT), np.float32)
    for p in range(N_PAIRS):
        base = p * TOK
        for j in range(2):
            r = res.results[2 * p + j]
            head_out[base:base + TOK, j * VHH:(j + 1) * VHH] = \
                r["head_o"].astype(np.float32)
            if len(idx0[p]):
                tail0[base + idx0[p], j * VTH:(j + 1) * VTH] = \
                    r["t0_o"][:len(idx0[p])].astype(np.float32)
            if len(idx1[p]):
                tail1[base + idx1[p], j * VTH:(j + 1) * VTH] = \
                    r["t1_o"][:len(idx1[p])].astype(np.float32)

    return (head_out, tail0, tail1, new_t_head, new_t0, new_t1)
